# revision 28
# baseline (speedup 1.0000x reference)
"""Multi-head attention (B=8, N=1024, D=768, 12 heads x 64) on 8 TRN2
NeuronCores, batch-parallel (one batch element per core, no collectives).

Per-core dataflow (everything transposed so no on-device transposes are
needed; x arrives host-transposed):
  - qkv projection as q^T,k^T (head-dim on partitions) and v (natural),
    fp32r matmuls at full PE rate
  - RoPE via a +-1 permutation matmul (rotate_half) + vector-engine
    elementwise combine against host-precomputed cos/sin tables
  - k^T stored zero-padded to 128 contraction rows per head: TRN2 matmuls
    with K<128 run ~2x slow, so S^T uses K=128 with the other head's q
    rows nulled by zero weights
  - S^T = k'.q'^T per head, exp on ScalarE (softmax scale folded into the
    activation's free affine), no max-subtraction (scores are O(5) here)
  - PV as out^T = [v|1]^T @ E^T -- the ones column yields the softmax
    denominators in psum row 64; normalization deferred past PV and
    pipelined per head-pair (reciprocal + broadcast-DMA + multiply)
  - out-projection from attnout^T with b_out folded in as a K=1 matmul;
    output is written transposed and untransposed on the host.
"""
import sys

sys.path.insert(0, "/opt/trn_rl_repo")

import numpy as np
import ml_dtypes

import concourse.bass as bass
import concourse.tile as tile_mod
from concourse import mybir
from concourse.bass_utils import run_bass_kernel_spmd
from concourse.vector_clock import ScopedClock

F32R = mybir.dt.float32r
F32 = mybir.dt.float32
BF16 = mybir.dt.bfloat16

B, N, D = 8, 1024, 768
H, DH = 12, 64
HP = H // 2          # head pairs (two heads share a 128-partition tile)
KC = D // 128        # contraction chunks for the projections
OC = 2 * KC          # q^T,k^T output-channel 128-tiles
RC = N // 128        # row chunks of the sequence
NK = N // 128        # key chunks
SCALE = DH ** -0.5


# --- walrus workaround: one sync-wait per instruction ---------------------
def _patched_drain_and_barrier(self, tick_clock, wait_clock):
    drain_inst = self.nc.sync.drain()
    wait_clock.add_sem_waits(
        drain_inst.ins, ScopedClock({None: tick_clock.global_clock})
    )
    si = drain_inst.ins.sync_info
    waits = list(si.on_wait or []) if si is not None else []
    if len(waits) > 1:
        drain_inst.ins.sync_info = mybir.SyncInfo(
            on_wait=waits[:1], on_update=list(si.on_update or [])
        )
        for w in waits[1:]:
            nop = self.nc.sync.nop(nofuse=True)
            nop.ins.sync_info = mybir.SyncInfo(on_wait=[w], on_update=[])
    self.nc.all_engine_barrier()
    assert self.sems is not None
    popped = self.nc._tile_sem_poison_stack.pop()
    assert popped is self._sem_poison
    self.nc.clear_and_free_semaphores(list(self.sems.allocated().values()))
    self.nc.all_engine_barrier()


tile_mod.TileContext._drain_and_barrier = _patched_drain_and_barrier



_split_counter = [0]


def split_sync_waits(nc, max_waits=1):
    """walrus rejects instructions carrying several sem waits; spill the
    excess onto engine-matched NOPs inserted directly before the offender."""
    for f in nc.m.functions:
        for bb in f.blocks:
            il = bb.instructions
            i = 0
            while i < len(il):
                inst = il[i]
                si = inst.sync_info
                waits = list(si.on_wait or []) if si is not None else []
                if len(waits) > max_waits:
                    inst.sync_info = mybir.SyncInfo(
                        on_wait=waits[:max_waits],
                        on_update=list(si.on_update or []),
                    )
                    rest = waits[max_waits:]
                    nops = []
                    for j in range(0, len(rest), max_waits):
                        _split_counter[0] += 1
                        nop = mybir.InstNoOp(
                            name=f"I-waitsplit-{_split_counter[0]}",
                            ins=[],
                            outs=[],
                            engine=inst.engine,
                        )
                        nop.sync_info = mybir.SyncInfo(
                            on_wait=rest[j : j + max_waits], on_update=[]
                        )
                        nops.append(nop)
                    for k, nop in enumerate(nops):
                        il.insert(i + k, nop)
                    i += len(nops)
                i += 1


def _bcast_rows(dram_ap, offset_elems, parts, free):
    """AP reading dram_ap[offset : offset+free] into `parts` partitions."""
    return bass.AP(
        tensor=dram_ap.tensor,
        offset=dram_ap.offset + offset_elems,
        ap=[[0, parts], [1, free]],
    )


def build_nc(with_bias=True):
    nc = bass.Bass()
    xt_d = nc.dram_tensor("xt", [D, N], BF16, kind="ExternalInput")
    wq_d = nc.dram_tensor("wq", [D, 3 * D], BF16, kind="ExternalInput")
    wo_d = nc.dram_tensor("wo", [D, D], BF16, kind="ExternalInput")
    bo_d = nc.dram_tensor("bo", [D], BF16, kind="ExternalInput")
    cos_d = nc.dram_tensor("cos2", [128, N], BF16, kind="ExternalInput")
    sin_d = nc.dram_tensor("sin2", [128, N], BF16, kind="ExternalInput")
    perm_d = nc.dram_tensor("perm", [128, 128], BF16, kind="ExternalInput")
    out_d = nc.dram_tensor("out", [D, N], F32, kind="ExternalOutput")
    import os as _os0

    _dbg = _os0.environ.get("K_DEBUG", "0") == "1"
    if _dbg:
        dbg_q = nc.dram_tensor("dbg_q", [128, KC, N], BF16, kind="ExternalOutput")
        dbg_k = nc.dram_tensor("dbg_k", [128, KC, N], BF16, kind="ExternalOutput")
        dbg_v = nc.dram_tensor(
            "dbg_v", [128, NK, H, DH + 1], BF16, kind="ExternalOutput"
        )
        dbg_au = nc.dram_tensor("dbg_au", [128, KC, N], F32, kind="ExternalOutput")
        dbg_sums = nc.dram_tensor("dbg_sums", [96, 128], F32, kind="ExternalOutput")
        dbg_attn = nc.dram_tensor("dbg_attn", [128, KC, N], BF16, kind="ExternalOutput")

    Exp = mybir.ActivationFunctionType.Exp
    Copy = mybir.ActivationFunctionType.Copy

    with tile_mod.TileContext(nc) as tc:
        with (
            tc.tile_pool(name="singles", bufs=1) as singles,
            tc.tile_pool(name="wq_pool", bufs=8) as wq_pool,
            tc.tile_pool(name="wo_pool", bufs=3) as wo_pool,
            tc.tile_pool(name="apool", bufs=3) as apool,
            tc.tile_pool(name="bpool", bufs=2) as bpool,
            tc.tile_pool(name="dpool", bufs=1, space="DRAM") as dpool,
        ):
            xt_sb = singles.tile([128, KC, N], BF16)
            wv_sb = singles.tile([128, KC, D], BF16)
            for kc in range(KC):
                nc.sync.dma_start(
                    out=xt_sb[:, kc, :], in_=xt_d[kc * 128 : (kc + 1) * 128, :]
                )
                nc.sync.dma_start(
                    out=wv_sb[:, kc, :],
                    in_=wq_d[kc * 128 : (kc + 1) * 128, 2 * D : 3 * D],
                )
            cos_sb = singles.tile([128, N], BF16)
            nc.sync.dma_start(out=cos_sb[:], in_=cos_d[:])
            sin_sb = singles.tile([128, N], BF16)
            nc.sync.dma_start(out=sin_sb[:], in_=sin_d[:])
            perm_sb = singles.tile([128, 128], BF16)
            nc.sync.dma_start(out=perm_sb[:], in_=perm_d[:])
            bo_sb = singles.tile([1, D], BF16)
            nc.sync.dma_start(
                out=bo_sb[:], in_=bo_d[:].rearrange("(o d) -> o d", o=1)
            )
            ones_sb = singles.tile([1, 512], BF16)
            nc.vector.memset(ones_sb[:], 1.0)
            wo_sb = singles.tile([128, KC, D], BF16)
            for c in range(KC):
                nc.sync.dma_start(
                    out=wo_sb[:, c, :], in_=wo_d[c * 128 : (c + 1) * 128, :]
                )

            v_sb = singles.tile([128, NK, H, DH + 1], BF16)
            nc.gpsimd.memset(v_sb[:, :, :, DH : DH + 1], 1.0)

            q_sb = singles.tile([128, KC, N], BF16)
            k_sb = singles.tile([128, KC, N], BF16)
            attnU_sb = singles.tile([128, KC, N], F32)
            attn_sb = singles.tile([128, KC, N], BF16)
            sums2_sb = singles.tile([96, 128], F32)
            recip2_sb = singles.tile([96, 128], F32)
            recip_d = dpool.tile([H * N], F32)
            recip_ap = recip_d[:]

            # ---- v projection: v[rows, 768] = x @ Wv -------------------
            with tc.tile_pool(name="ps_v", bufs=2, space="PSUM") as ps_v:
                for rc in range(RC):
                    vp = ps_v.tile([128, D], F32, tag="v")
                    for c0, w in ((0, 512), (512, 256)):
                        for kc in range(KC):
                            nc.tensor.matmul(
                                vp[:, c0 : c0 + w],
                                xt_sb[:, kc, rc * 128 : (rc + 1) * 128],
                                wv_sb[:, kc, c0 : c0 + w],
                                start=(kc == 0),
                                stop=(kc == KC - 1),
                            )
                    # strided copy into the [v | ones] per-head layout
                    nc.scalar.activation(
                        out=v_sb[:, rc, :, 0:DH],
                        in_=vp[:].rearrange("p (h d) -> p h d", h=H),
                        func=Copy,
                        scale=1.0,
                    )

            # ---- q^T / k^T projection + RoPE (one 128-col tile) --------
            def proj_oc(ps_qk, oc):
                col0 = oc * 128 if oc < KC else D + (oc - KC) * 128
                qkp = ps_qk.tile([128, N], F32, tag="qk", name=f"qkp{oc}")
                wts = []
                for kc in range(KC):
                    wt = wq_pool.tile([128, 128], BF16, tag="wq", name=f"wt{oc}_{kc}")
                    nc.sync.dma_start(
                        out=wt[:],
                        in_=wq_d[kc * 128 : (kc + 1) * 128, col0 : col0 + 128],
                    )
                    wts.append(wt)
                for qc in range(2):
                    for kc in range(KC):
                        nc.tensor.matmul(
                            qkp[:, qc * 512 : (qc + 1) * 512],
                            wts[kc][:],
                            xt_sb[:, kc, qc * 512 : (qc + 1) * 512],
                            start=(kc == 0),
                            stop=(kc == KC - 1),
                        )
                q0 = apool.tile([128, N], BF16, tag="q0", name=f"q0_{oc}")
                nc.vector.tensor_copy(q0[:], qkp[:])
                rotp = ps_qk.tile([128, N], F32, tag="qk", name=f"rotp{oc}")
                for qc in range(2):
                    nc.tensor.matmul(
                        rotp[:, qc * 512 : (qc + 1) * 512],
                        perm_sb[:],
                        q0[:, qc * 512 : (qc + 1) * 512],
                        start=True,
                        stop=True,
                    )
                t1 = apool.tile([128, N], BF16, tag="t1", name=f"t1_{oc}")
                nc.vector.tensor_mul(t1[:], rotp[:], sin_sb[:])
                t2 = apool.tile([128, N], BF16, tag="t2", name=f"t2_{oc}")
                nc.vector.tensor_mul(t2[:], q0[:], cos_sb[:])
                dst = q_sb if oc < KC else k_sb
                nc.vector.tensor_add(dst[:, oc % KC, :], t1[:], t2[:])

            # ---- attention: head pairs, row-packed K=64 S^T matmuls,
            # kc-paired psum tiles for wide exp, query-split for psum room
            def attn_pair(ps_att, qc, hp):
                # st tile per kc holds BOTH heads' scores side by side:
                # [128 keys, (head_a 512q | head_b 512q)] -> one wide exp
                pvs = []
                for a in range(2):
                    pv = ps_att.tile(
                        [65, 512], F32, tag=f"pv{a}", bufs=1, name=f"pv{a}_{qc}_{hp}"
                    )
                    pvs.append(pv)
                for kc in range(NK):
                    st = ps_att.tile(
                        [128, N], F32, tag="st", bufs=2, name=f"st_{qc}_{hp}_{kc}"
                    )
                    for a in range(2):
                        po = 64 * a
                        nc.tensor.matmul(
                            st[:, a * 512 : (a + 1) * 512],
                            k_sb[po : po + 64, hp, kc * 128 : (kc + 1) * 128],
                            q_sb[po : po + 64, hp, qc * 512 : (qc + 1) * 512],
                            start=True,
                            stop=True,
                        )
                    e = apool.tile([128, N], BF16, tag="e", name=f"e_{qc}_{hp}_{kc}")
                    nc.scalar.activation(out=e[:], in_=st[:], func=Exp, scale=SCALE)
                    for a in range(2):
                        nc.tensor.matmul(
                            pvs[a][:],
                            v_sb[:, kc, 2 * hp + a, :],
                            e[:, a * 512 : (a + 1) * 512],
                            start=(kc == 0),
                            stop=(kc == NK - 1),
                        )
                for a in range(2):
                    h = 2 * hp + a
                    po = 64 * a
                    pvt = bpool.tile(
                        [65, 512], F32, tag="pvt", name=f"pvt{qc}_{h}"
                    )
                    nc.vector.tensor_copy(pvt[:], pvs[a][:])
                    nc.sync.dma_start(
                        out=attnU_sb[po : po + 64, hp, qc * 512 : (qc + 1) * 512],
                        in_=pvt[0:64, :],
                    )
                    nc.sync.dma_start(
                        out=sums2_sb[h * 8 + qc * 4 : h * 8 + qc * 4 + 4, :],
                        in_=pvt[64:65, :],
                    )

            def normalize_quad(q4):
                nc.vector.reciprocal(
                    recip2_sb[q4 * 32 : (q4 + 1) * 32, :],
                    sums2_sb[q4 * 32 : (q4 + 1) * 32, :],
                )
                nc.sync.dma_start(
                    out=recip_d[:].rearrange("(p r) -> p r", p=96)[
                        q4 * 32 : (q4 + 1) * 32, :
                    ],
                    in_=recip2_sb[q4 * 32 : (q4 + 1) * 32, :],
                )
                for hp2 in (2 * q4, 2 * q4 + 1):
                    rb = bpool.tile([128, N], F32, tag="rb", name=f"rb{hp2}")
                    nc.sync.dma_start(
                        out=rb[0:64, :],
                        in_=_bcast_rows(recip_ap, (2 * hp2) * N, 64, N),
                    )
                    nc.sync.dma_start(
                        out=rb[64:128, :],
                        in_=_bcast_rows(recip_ap, (2 * hp2 + 1) * N, 64, N),
                    )
                    nc.vector.tensor_mul(
                        attn_sb[:, hp2, :], attnU_sb[:, hp2, :], rb[:]
                    )

            import os as _os

            _mode = _os.environ.get("K_MODE", "inter")
            if _mode == "v3a":
                # proj fully upfront in its own psum pool, then attention
                with tc.tile_pool(name="ps_qk", bufs=1, space="PSUM") as ps_qk:
                    for oc in range(KC):
                        proj_oc(ps_qk, oc)
                        proj_oc(ps_qk, KC + oc)
                with tc.tile_pool(name="ps_att", bufs=1, space="PSUM") as ps_att:
                    for hp in range(HP):
                        attn_pair(ps_att, 0, hp)
                    for hp in range(HP):
                        attn_pair(ps_att, 1, hp)
                        if hp % 2 == 1:
                            normalize_quad(hp // 2)
            else:
                with (
                    tc.tile_pool(name="ps_qk", bufs=1, space="PSUM") as ps_qk,
                    tc.tile_pool(name="ps_att", bufs=1, space="PSUM") as ps_att,
                ):
                    proj_oc(ps_qk, 0)
                    proj_oc(ps_qk, KC)
                    for hp in range(HP):
                        attn_pair(ps_att, 0, hp)
                        if hp + 1 < HP:
                            proj_oc(ps_qk, hp + 1)
                            proj_oc(ps_qk, KC + hp + 1)
                    for hp in range(HP):
                        attn_pair(ps_att, 1, hp)
                        if hp % 2 == 1:
                            normalize_quad(hp // 2)

            if _dbg:
                nc.sync.dma_start(out=dbg_q[:], in_=q_sb[:])
                nc.sync.dma_start(out=dbg_k[:], in_=k_sb[:])
                nc.sync.dma_start(out=dbg_v[:], in_=v_sb[:])
                nc.sync.dma_start(out=dbg_au[:], in_=attnU_sb[:])
                nc.sync.dma_start(out=dbg_sums[:], in_=sums2_sb[:])
                nc.sync.dma_start(out=dbg_attn[:], in_=attn_sb[:])

            # ---- out-projection ----------------------------------------
            with tc.tile_pool(name="ps_fin", bufs=3, space="PSUM") as ps_fin:
                for oc in range(KC):
                    fps = [
                        ps_fin.tile([128, 512], F32, tag="fin", name=f"fin{oc}_{i}")
                        for i in range(2)
                    ]
                    for c in range(KC):
                        for qc in range(2):
                            nc.tensor.matmul(
                                fps[qc][:],
                                wo_sb[:, c, oc * 128 : (oc + 1) * 128],
                                attn_sb[:, c, qc * 512 : (qc + 1) * 512],
                                start=(c == 0),
                                stop=(not with_bias and c == KC - 1),
                            )
                    for qc in range(2):
                        if with_bias:
                            nc.tensor.matmul(
                                fps[qc][:],
                                bo_sb[0:1, oc * 128 : (oc + 1) * 128],
                                ones_sb[:],
                                start=False,
                                stop=True,
                            )
                        fsb = bpool.tile([128, 512], F32, tag="fsb")
                        nc.scalar.activation(
                            out=fsb[:], in_=fps[qc][:], func=Copy, scale=1.0
                        )
                        nc.sync.dma_start(
                            out=out_d[
                                oc * 128 : (oc + 1) * 128, qc * 512 : (qc + 1) * 512
                            ],
                            in_=fsb[:],
                        )

    split_sync_waits(nc, max_waits=1)
    return nc


def _host_prep(x, w_qkv, w_out, b_out):
    bf = ml_dtypes.bfloat16
    inv_freq = 1.0 / (10000.0 ** (np.arange(0, DH, 2, dtype=np.float32) / DH))
    t = np.arange(N, dtype=np.float32)
    freqs = np.outer(t, inv_freq)
    emb = np.concatenate([freqs, freqs], axis=1)        # [N, DH]
    cos2 = np.tile(np.cos(emb).T.astype(np.float32), (2, 1)).astype(bf)
    sin2 = np.tile(np.sin(emb).T.astype(np.float32), (2, 1)).astype(bf)

    perm = np.zeros((128, 128), np.float32)
    for blk in range(2):
        o = blk * 64
        for m in range(32):
            perm[o + m + 32, o + m] = -1.0
        for m in range(32, 64):
            perm[o + m - 32, o + m] = 1.0
    perm = perm.astype(bf)

    xt = np.ascontiguousarray(x.transpose(0, 2, 1)).astype(bf)
    shared = {
        "wq": np.ascontiguousarray(w_qkv).astype(bf),
        "wo": np.ascontiguousarray(w_out).astype(bf),
        "bo": np.ascontiguousarray(b_out).astype(bf),
        "cos2": np.ascontiguousarray(cos2),
        "sin2": np.ascontiguousarray(sin2),
        "perm": np.ascontiguousarray(perm),
    }
    return [dict(shared, xt=np.ascontiguousarray(xt[i])) for i in range(B)]


_NC_CACHE = {}
LAST_EXEC_NS = [None]


def _run(in_maps, trace=False, with_bias=True):
    if with_bias not in _NC_CACHE:
        _NC_CACHE[with_bias] = build_nc(with_bias=with_bias)
    res = run_bass_kernel_spmd(
        _NC_CACHE[with_bias], in_maps, list(range(B)), trace=trace
    )
    LAST_EXEC_NS[0] = res.exec_time_ns
    out_t = np.stack([np.asarray(res.results[i]["out"]) for i in range(B)])
    return np.ascontiguousarray(out_t.transpose(0, 2, 1)).astype(np.float32)


def kernel(x, w_qkv, w_out, b_out, _trace=False):
    b_out = np.asarray(b_out, dtype=np.float32)
    in_maps = _host_prep(
        np.asarray(x, dtype=np.float32),
        np.asarray(w_qkv, dtype=np.float32),
        np.asarray(w_out, dtype=np.float32),
        b_out,
    )
    return _run(in_maps, trace=_trace, with_bias=bool(np.any(b_out)))


# revision 30
# speedup vs baseline: 1.0132x; 1.0132x over previous
"""Multi-head attention (B=8, N=1024, D=768, 12 heads x 64) on 8 TRN2
NeuronCores, batch-parallel (one batch element per core, no collectives).

Per-core dataflow (everything transposed so no on-device transposes are
needed; x arrives host-transposed):
  - qkv projection as q^T,k^T (head-dim on partitions) and v (natural),
    fp32r matmuls at full PE rate
  - RoPE via a +-1 permutation matmul (rotate_half) + vector-engine
    elementwise combine against host-precomputed cos/sin tables
  - k^T stored zero-padded to 128 contraction rows per head: TRN2 matmuls
    with K<128 run ~2x slow, so S^T uses K=128 with the other head's q
    rows nulled by zero weights
  - S^T = k'.q'^T per head, exp on ScalarE (softmax scale folded into the
    activation's free affine), no max-subtraction (scores are O(5) here)
  - PV as out^T = [v|1]^T @ E^T -- the ones column yields the softmax
    denominators in psum row 64; normalization deferred past PV and
    pipelined per head-pair (reciprocal + broadcast-DMA + multiply)
  - out-projection from attnout^T with b_out folded in as a K=1 matmul;
    output is written transposed and untransposed on the host.
"""
import sys

sys.path.insert(0, "/opt/trn_rl_repo")

import numpy as np
import ml_dtypes

import concourse.bass as bass
import concourse.tile as tile_mod
from concourse import mybir
from concourse.bass_utils import run_bass_kernel_spmd
from concourse.vector_clock import ScopedClock

F32R = mybir.dt.float32r
F32 = mybir.dt.float32
BF16 = mybir.dt.bfloat16

B, N, D = 8, 1024, 768
H, DH = 12, 64
HP = H // 2          # head pairs (two heads share a 128-partition tile)
KC = D // 128        # contraction chunks for the projections
OC = 2 * KC          # q^T,k^T output-channel 128-tiles
RC = N // 128        # row chunks of the sequence
NK = N // 128        # key chunks
SCALE = DH ** -0.5


# --- walrus workaround: one sync-wait per instruction ---------------------
def _patched_drain_and_barrier(self, tick_clock, wait_clock):
    drain_inst = self.nc.sync.drain()
    wait_clock.add_sem_waits(
        drain_inst.ins, ScopedClock({None: tick_clock.global_clock})
    )
    si = drain_inst.ins.sync_info
    waits = list(si.on_wait or []) if si is not None else []
    if len(waits) > 1:
        drain_inst.ins.sync_info = mybir.SyncInfo(
            on_wait=waits[:1], on_update=list(si.on_update or [])
        )
        for w in waits[1:]:
            nop = self.nc.sync.nop(nofuse=True)
            nop.ins.sync_info = mybir.SyncInfo(on_wait=[w], on_update=[])
    self.nc.all_engine_barrier()
    assert self.sems is not None
    popped = self.nc._tile_sem_poison_stack.pop()
    assert popped is self._sem_poison
    self.nc.clear_and_free_semaphores(list(self.sems.allocated().values()))
    self.nc.all_engine_barrier()


tile_mod.TileContext._drain_and_barrier = _patched_drain_and_barrier



_split_counter = [0]


def split_sync_waits(nc, max_waits=1):
    """walrus rejects instructions carrying several sem waits; spill the
    excess onto engine-matched NOPs inserted directly before the offender."""
    for f in nc.m.functions:
        for bb in f.blocks:
            il = bb.instructions
            i = 0
            while i < len(il):
                inst = il[i]
                si = inst.sync_info
                waits = list(si.on_wait or []) if si is not None else []
                if len(waits) > max_waits:
                    inst.sync_info = mybir.SyncInfo(
                        on_wait=waits[:max_waits],
                        on_update=list(si.on_update or []),
                    )
                    rest = waits[max_waits:]
                    nops = []
                    for j in range(0, len(rest), max_waits):
                        _split_counter[0] += 1
                        nop = mybir.InstNoOp(
                            name=f"I-waitsplit-{_split_counter[0]}",
                            ins=[],
                            outs=[],
                            engine=inst.engine,
                        )
                        nop.sync_info = mybir.SyncInfo(
                            on_wait=rest[j : j + max_waits], on_update=[]
                        )
                        nops.append(nop)
                    for k, nop in enumerate(nops):
                        il.insert(i + k, nop)
                    i += len(nops)
                i += 1


def _bcast_rows(dram_ap, offset_elems, parts, free):
    """AP reading dram_ap[offset : offset+free] into `parts` partitions."""
    return bass.AP(
        tensor=dram_ap.tensor,
        offset=dram_ap.offset + offset_elems,
        ap=[[0, parts], [1, free]],
    )


def build_nc(with_bias=True):
    nc = bass.Bass()
    xt_d = nc.dram_tensor("xt", [D, N], BF16, kind="ExternalInput")
    wq_d = nc.dram_tensor("wq", [D, 3 * D], BF16, kind="ExternalInput")
    wo_d = nc.dram_tensor("wo", [D, D], BF16, kind="ExternalInput")
    bo_d = nc.dram_tensor("bo", [D], BF16, kind="ExternalInput")
    cos_d = nc.dram_tensor("cos2", [128, N], BF16, kind="ExternalInput")
    sin_d = nc.dram_tensor("sin2", [128, N], BF16, kind="ExternalInput")
    perm_d = nc.dram_tensor("perm", [128, 128], BF16, kind="ExternalInput")
    out_d = nc.dram_tensor("out", [D, N], F32, kind="ExternalOutput")
    import os as _os0

    _dbg = _os0.environ.get("K_DEBUG", "0") == "1"
    if _dbg:
        dbg_q = nc.dram_tensor("dbg_q", [128, KC, N], BF16, kind="ExternalOutput")
        dbg_k = nc.dram_tensor("dbg_k", [128, KC, N], BF16, kind="ExternalOutput")
        dbg_v = nc.dram_tensor(
            "dbg_v", [128, NK, H, DH + 1], BF16, kind="ExternalOutput"
        )
        dbg_au = nc.dram_tensor("dbg_au", [128, KC, N], F32, kind="ExternalOutput")
        dbg_sums = nc.dram_tensor("dbg_sums", [96, 128], F32, kind="ExternalOutput")
        dbg_attn = nc.dram_tensor("dbg_attn", [128, KC, N], BF16, kind="ExternalOutput")

    Exp = mybir.ActivationFunctionType.Exp
    Copy = mybir.ActivationFunctionType.Copy

    with tile_mod.TileContext(nc) as tc:
        with (
            tc.tile_pool(name="singles", bufs=1) as singles,
            tc.tile_pool(name="wq_pool", bufs=8) as wq_pool,
            tc.tile_pool(name="wo_pool", bufs=3) as wo_pool,
            tc.tile_pool(name="apool", bufs=3) as apool,
            tc.tile_pool(name="bpool", bufs=2) as bpool,
            tc.tile_pool(name="dpool", bufs=1, space="DRAM") as dpool,
        ):
            xt_sb = singles.tile([128, KC, N], BF16)
            wv_sb = singles.tile([128, KC, D], BF16)
            for kc in range(KC):
                nc.sync.dma_start(
                    out=xt_sb[:, kc, :], in_=xt_d[kc * 128 : (kc + 1) * 128, :]
                )
                nc.sync.dma_start(
                    out=wv_sb[:, kc, :],
                    in_=wq_d[kc * 128 : (kc + 1) * 128, 2 * D : 3 * D],
                )
            cos_sb = singles.tile([128, N], BF16)
            nc.sync.dma_start(out=cos_sb[:], in_=cos_d[:])
            sin_sb = singles.tile([128, N], BF16)
            nc.sync.dma_start(out=sin_sb[:], in_=sin_d[:])
            perm_sb = singles.tile([128, 128], BF16)
            nc.sync.dma_start(out=perm_sb[:], in_=perm_d[:])
            bo_sb = singles.tile([1, D], BF16)
            nc.sync.dma_start(
                out=bo_sb[:], in_=bo_d[:].rearrange("(o d) -> o d", o=1)
            )
            ones_sb = singles.tile([1, 512], BF16)
            nc.vector.memset(ones_sb[:], 1.0)
            wo_sb = singles.tile([128, KC, D], BF16)
            for c in range(KC):
                nc.sync.dma_start(
                    out=wo_sb[:, c, :], in_=wo_d[c * 128 : (c + 1) * 128, :]
                )

            v_sb = singles.tile([128, NK, H, DH + 1], BF16)
            nc.gpsimd.memset(v_sb[:, :, :, DH : DH + 1], 1.0)

            q_sb = singles.tile([128, KC, N], BF16)
            k_sb = singles.tile([128, KC, N], BF16)
            attnU_sb = singles.tile([128, KC, N], F32)
            attn_sb = singles.tile([128, KC, N], BF16)
            sums2_sb = singles.tile([96, 128], F32)
            recip2_sb = singles.tile([96, 128], F32)
            recip_d = dpool.tile([H * N], F32)
            recip_ap = recip_d[:]

            # ---- v projection: v[rows, 768] = x @ Wv -------------------
            with tc.tile_pool(name="ps_v", bufs=2, space="PSUM") as ps_v:
                for rc in range(RC):
                    vp = ps_v.tile([128, D], F32, tag="v")
                    for c0, w in ((0, 512), (512, 256)):
                        for kc in range(KC):
                            nc.tensor.matmul(
                                vp[:, c0 : c0 + w],
                                xt_sb[:, kc, rc * 128 : (rc + 1) * 128],
                                wv_sb[:, kc, c0 : c0 + w],
                                start=(kc == 0),
                                stop=(kc == KC - 1),
                            )
                    # strided copy into the [v | ones] per-head layout
                    nc.scalar.activation(
                        out=v_sb[:, rc, :, 0:DH],
                        in_=vp[:].rearrange("p (h d) -> p h d", h=H),
                        func=Copy,
                        scale=1.0,
                    )

            # ---- q^T / k^T projection + RoPE (one 128-col tile) --------
            def proj_oc(ps_qk, oc):
                col0 = oc * 128 if oc < KC else D + (oc - KC) * 128
                qkp = ps_qk.tile([128, N], F32, tag="qk", name=f"qkp{oc}")
                wts = []
                for kc in range(KC):
                    wt = wq_pool.tile([128, 128], BF16, tag="wq", name=f"wt{oc}_{kc}")
                    nc.sync.dma_start(
                        out=wt[:],
                        in_=wq_d[kc * 128 : (kc + 1) * 128, col0 : col0 + 128],
                    )
                    wts.append(wt)
                for qc in range(2):
                    for kc in range(KC):
                        nc.tensor.matmul(
                            qkp[:, qc * 512 : (qc + 1) * 512],
                            wts[kc][:],
                            xt_sb[:, kc, qc * 512 : (qc + 1) * 512],
                            start=(kc == 0),
                            stop=(kc == KC - 1),
                        )
                q0 = apool.tile([128, N], BF16, tag="q0", name=f"q0_{oc}")
                nc.vector.tensor_copy(q0[:], qkp[:])
                rotp = ps_qk.tile([128, N], F32, tag="qk", name=f"rotp{oc}")
                for qc in range(2):
                    nc.tensor.matmul(
                        rotp[:, qc * 512 : (qc + 1) * 512],
                        perm_sb[:],
                        q0[:, qc * 512 : (qc + 1) * 512],
                        start=True,
                        stop=True,
                    )
                t1 = apool.tile([128, N], BF16, tag="t1", name=f"t1_{oc}")
                nc.vector.tensor_mul(t1[:], rotp[:], sin_sb[:])
                t2 = apool.tile([128, N], BF16, tag="t2", name=f"t2_{oc}")
                nc.vector.tensor_mul(t2[:], q0[:], cos_sb[:])
                dst = q_sb if oc < KC else k_sb
                nc.vector.tensor_add(dst[:, oc % KC, :], t1[:], t2[:])

            # ---- attention: head pairs, row-packed K=64 S^T matmuls,
            # kc-paired psum tiles for wide exp, query-split for psum room
            def attn_pair(ps_att, qc, hp):
                # st tile per kc holds BOTH heads' scores side by side:
                # [128 keys, (head_a 512q | head_b 512q)] -> one wide exp
                pvs = []
                for a in range(2):
                    pv = ps_att.tile(
                        [65, 512], F32, tag=f"pv{a}", bufs=1, name=f"pv{a}_{qc}_{hp}"
                    )
                    pvs.append(pv)
                for kc in range(NK):
                    st = ps_att.tile(
                        [128, N], F32, tag="st", bufs=2, name=f"st_{qc}_{hp}_{kc}"
                    )
                    for a in range(2):
                        po = 64 * a
                        nc.tensor.matmul(
                            st[:, a * 512 : (a + 1) * 512],
                            k_sb[po : po + 64, hp, kc * 128 : (kc + 1) * 128],
                            q_sb[po : po + 64, hp, qc * 512 : (qc + 1) * 512],
                            start=True,
                            stop=True,
                        )
                    e = apool.tile([128, N], BF16, tag="e", name=f"e_{qc}_{hp}_{kc}")
                    nc.scalar.activation(out=e[:], in_=st[:], func=Exp, scale=SCALE)
                    for a in range(2):
                        nc.tensor.matmul(
                            pvs[a][:],
                            v_sb[:, kc, 2 * hp + a, :],
                            e[:, a * 512 : (a + 1) * 512],
                            start=(kc == 0),
                            stop=(kc == NK - 1),
                        )
                for a in range(2):
                    h = 2 * hp + a
                    po = 64 * a
                    pvt = bpool.tile(
                        [65, 512], F32, tag="pvt", name=f"pvt{qc}_{h}"
                    )
                    nc.vector.tensor_copy(pvt[:], pvs[a][:])
                    nc.sync.dma_start(
                        out=attnU_sb[po : po + 64, hp, qc * 512 : (qc + 1) * 512],
                        in_=pvt[0:64, :],
                    )
                    nc.sync.dma_start(
                        out=sums2_sb[h * 8 + qc * 4 : h * 8 + qc * 4 + 4, :],
                        in_=pvt[64:65, :],
                    )

            def normalize_quad(q4):
                nc.vector.reciprocal(
                    recip2_sb[q4 * 32 : (q4 + 1) * 32, :],
                    sums2_sb[q4 * 32 : (q4 + 1) * 32, :],
                )
                nc.sync.dma_start(
                    out=recip_d[:].rearrange("(p r) -> p r", p=96)[
                        q4 * 32 : (q4 + 1) * 32, :
                    ],
                    in_=recip2_sb[q4 * 32 : (q4 + 1) * 32, :],
                )
                for hp2 in (2 * q4, 2 * q4 + 1):
                    rb = bpool.tile([128, N], F32, tag="rb", name=f"rb{hp2}")
                    nc.sync.dma_start(
                        out=rb[0:64, :],
                        in_=_bcast_rows(recip_ap, (2 * hp2) * N, 64, N),
                    )
                    nc.sync.dma_start(
                        out=rb[64:128, :],
                        in_=_bcast_rows(recip_ap, (2 * hp2 + 1) * N, 64, N),
                    )
                    nc.vector.tensor_mul(
                        attn_sb[:, hp2, :], attnU_sb[:, hp2, :], rb[:]
                    )

            import os as _os

            _mode = _os.environ.get("K_MODE", "inter")
            if _mode == "v3a":
                # proj fully upfront in its own psum pool, then attention
                with tc.tile_pool(name="ps_qk", bufs=1, space="PSUM") as ps_qk:
                    for oc in range(KC):
                        proj_oc(ps_qk, oc)
                        proj_oc(ps_qk, KC + oc)
                with tc.tile_pool(name="ps_att", bufs=1, space="PSUM") as ps_att:
                    for hp in range(HP):
                        attn_pair(ps_att, 0, hp)
                    for hp in range(HP):
                        attn_pair(ps_att, 1, hp)
                        if hp % 2 == 1:
                            normalize_quad(hp // 2)
            else:
                with (
                    tc.tile_pool(name="ps_qk", bufs=1, space="PSUM") as ps_qk,
                    tc.tile_pool(name="ps_att", bufs=1, space="PSUM") as ps_att,
                ):
                    proj_oc(ps_qk, 0)
                    proj_oc(ps_qk, KC)
                    for hp in range(HP):
                        attn_pair(ps_att, 0, hp)
                        if hp + 1 < HP:
                            proj_oc(ps_qk, hp + 1)
                            proj_oc(ps_qk, KC + hp + 1)
                    for hp in range(HP):
                        attn_pair(ps_att, 1, hp)
                        if hp % 2 == 1:
                            normalize_quad(hp // 2)

            if _dbg:
                nc.sync.dma_start(out=dbg_q[:], in_=q_sb[:])
                nc.sync.dma_start(out=dbg_k[:], in_=k_sb[:])
                nc.sync.dma_start(out=dbg_v[:], in_=v_sb[:])
                nc.sync.dma_start(out=dbg_au[:], in_=attnU_sb[:])
                nc.sync.dma_start(out=dbg_sums[:], in_=sums2_sb[:])
                nc.sync.dma_start(out=dbg_attn[:], in_=attn_sb[:])

            # ---- out-projection ----------------------------------------
            with tc.tile_pool(name="ps_fin", bufs=3, space="PSUM") as ps_fin:
                for oc in range(KC):
                    fps = [
                        ps_fin.tile([128, 512], F32, tag="fin", name=f"fin{oc}_{i}")
                        for i in range(2)
                    ]
                    for c in range(KC):
                        for qc in range(2):
                            nc.tensor.matmul(
                                fps[qc][:],
                                wo_sb[:, c, oc * 128 : (oc + 1) * 128],
                                attn_sb[:, c, qc * 512 : (qc + 1) * 512],
                                start=(c == 0),
                                stop=(not with_bias and c == KC - 1),
                            )
                    for qc in range(2):
                        if with_bias:
                            nc.tensor.matmul(
                                fps[qc][:],
                                bo_sb[0:1, oc * 128 : (oc + 1) * 128],
                                ones_sb[:],
                                start=False,
                                stop=True,
                            )
                        fsb = bpool.tile([128, 512], F32, tag="fsb")
                        nc.scalar.activation(
                            out=fsb[:], in_=fps[qc][:], func=Copy, scale=1.0
                        )
                        nc.sync.dma_start(
                            out=out_d[
                                oc * 128 : (oc + 1) * 128, qc * 512 : (qc + 1) * 512
                            ],
                            in_=fsb[:],
                        )

    split_sync_waits(nc, max_waits=1)
    return nc


def _host_prep(x, w_qkv, w_out, b_out):
    bf = ml_dtypes.bfloat16
    inv_freq = 1.0 / (10000.0 ** (np.arange(0, DH, 2, dtype=np.float32) / DH))
    t = np.arange(N, dtype=np.float32)
    freqs = np.outer(t, inv_freq)
    emb = np.concatenate([freqs, freqs], axis=1)        # [N, DH]
    cos2 = np.tile(np.cos(emb).T.astype(np.float32), (2, 1)).astype(bf)
    sin2 = np.tile(np.sin(emb).T.astype(np.float32), (2, 1)).astype(bf)

    perm = np.zeros((128, 128), np.float32)
    for blk in range(2):
        o = blk * 64
        for m in range(32):
            perm[o + m + 32, o + m] = -1.0
        for m in range(32, 64):
            perm[o + m - 32, o + m] = 1.0
    perm = perm.astype(bf)

    xt = np.ascontiguousarray(x.transpose(0, 2, 1)).astype(bf)
    shared = {
        "wq": np.ascontiguousarray(w_qkv).astype(bf),
        "wo": np.ascontiguousarray(w_out).astype(bf),
        "bo": np.ascontiguousarray(b_out).astype(bf),
        "cos2": np.ascontiguousarray(cos2),
        "sin2": np.ascontiguousarray(sin2),
        "perm": np.ascontiguousarray(perm),
    }
    return [dict(shared, xt=np.ascontiguousarray(xt[i])) for i in range(B)]


_NC_CACHE = {}
LAST_EXEC_NS = [None]


def _run(in_maps, trace=False, with_bias=True):
    if with_bias not in _NC_CACHE:
        _NC_CACHE[with_bias] = build_nc(with_bias=with_bias)
    res = run_bass_kernel_spmd(
        _NC_CACHE[with_bias], in_maps, list(range(B)), trace=trace
    )
    LAST_EXEC_NS[0] = res.exec_time_ns
    out_t = np.stack([np.asarray(res.results[i]["out"]) for i in range(B)])
    return np.ascontiguousarray(out_t.transpose(0, 2, 1)).astype(np.float32)


def kernel(x, w_qkv, w_out, b_out, _trace=False):
    b_out = np.asarray(b_out, dtype=np.float32)
    in_maps = _host_prep(
        np.asarray(x, dtype=np.float32),
        np.asarray(w_qkv, dtype=np.float32),
        np.asarray(w_out, dtype=np.float32),
        b_out,
    )
    return _run(in_maps, trace=_trace, with_bias=bool(np.any(b_out)))


# revision 31
# speedup vs baseline: 1.0492x; 1.0355x over previous
"""Multi-head attention (B=8, N=1024, D=768, 12 heads x 64) on 8 TRN2
NeuronCores, batch-parallel (one batch element per core, no collectives).

Per-core dataflow (everything transposed so no on-device transposes are
needed; x arrives host-transposed):
  - qkv projection as q^T,k^T (head-dim on partitions) and v (natural),
    fp32r matmuls at full PE rate
  - RoPE via a +-1 permutation matmul (rotate_half) + vector-engine
    elementwise combine against host-precomputed cos/sin tables
  - k^T stored zero-padded to 128 contraction rows per head: TRN2 matmuls
    with K<128 run ~2x slow, so S^T uses K=128 with the other head's q
    rows nulled by zero weights
  - S^T = k'.q'^T per head, exp on ScalarE (softmax scale folded into the
    activation's free affine), no max-subtraction (scores are O(5) here)
  - PV as out^T = [v|1]^T @ E^T -- the ones column yields the softmax
    denominators in psum row 64; normalization deferred past PV and
    pipelined per head-pair (reciprocal + broadcast-DMA + multiply)
  - out-projection from attnout^T with b_out folded in as a K=1 matmul;
    output is written transposed and untransposed on the host.
"""
import sys

sys.path.insert(0, "/opt/trn_rl_repo")

import numpy as np
import ml_dtypes

import concourse.bass as bass
import concourse.tile as tile_mod
from concourse import mybir
from concourse.bass_utils import run_bass_kernel_spmd
from concourse.vector_clock import ScopedClock

F32R = mybir.dt.float32r
F32 = mybir.dt.float32
BF16 = mybir.dt.bfloat16

B, N, D = 8, 1024, 768
H, DH = 12, 64
HP = H // 2          # head pairs (two heads share a 128-partition tile)
KC = D // 128        # contraction chunks for the projections
OC = 2 * KC          # q^T,k^T output-channel 128-tiles
RC = N // 128        # row chunks of the sequence
NK = N // 128        # key chunks
SCALE = DH ** -0.5


# --- walrus workaround: one sync-wait per instruction ---------------------
def _patched_drain_and_barrier(self, tick_clock, wait_clock):
    drain_inst = self.nc.sync.drain()
    wait_clock.add_sem_waits(
        drain_inst.ins, ScopedClock({None: tick_clock.global_clock})
    )
    si = drain_inst.ins.sync_info
    waits = list(si.on_wait or []) if si is not None else []
    if len(waits) > 1:
        drain_inst.ins.sync_info = mybir.SyncInfo(
            on_wait=waits[:1], on_update=list(si.on_update or [])
        )
        for w in waits[1:]:
            nop = self.nc.sync.nop(nofuse=True)
            nop.ins.sync_info = mybir.SyncInfo(on_wait=[w], on_update=[])
    self.nc.all_engine_barrier()
    assert self.sems is not None
    popped = self.nc._tile_sem_poison_stack.pop()
    assert popped is self._sem_poison
    self.nc.clear_and_free_semaphores(list(self.sems.allocated().values()))
    self.nc.all_engine_barrier()


tile_mod.TileContext._drain_and_barrier = _patched_drain_and_barrier



_split_counter = [0]


def split_sync_waits(nc, max_waits=1):
    """walrus rejects instructions carrying several sem waits; spill the
    excess onto engine-matched NOPs inserted directly before the offender."""
    for f in nc.m.functions:
        for bb in f.blocks:
            il = bb.instructions
            i = 0
            while i < len(il):
                inst = il[i]
                si = inst.sync_info
                waits = list(si.on_wait or []) if si is not None else []
                if len(waits) > max_waits:
                    inst.sync_info = mybir.SyncInfo(
                        on_wait=waits[:max_waits],
                        on_update=list(si.on_update or []),
                    )
                    rest = waits[max_waits:]
                    nops = []
                    for j in range(0, len(rest), max_waits):
                        _split_counter[0] += 1
                        nop = mybir.InstNoOp(
                            name=f"I-waitsplit-{_split_counter[0]}",
                            ins=[],
                            outs=[],
                            engine=inst.engine,
                        )
                        nop.sync_info = mybir.SyncInfo(
                            on_wait=rest[j : j + max_waits], on_update=[]
                        )
                        nops.append(nop)
                    for k, nop in enumerate(nops):
                        il.insert(i + k, nop)
                    i += len(nops)
                i += 1


def _bcast_rows(dram_ap, offset_elems, parts, free):
    """AP reading dram_ap[offset : offset+free] into `parts` partitions."""
    return bass.AP(
        tensor=dram_ap.tensor,
        offset=dram_ap.offset + offset_elems,
        ap=[[0, parts], [1, free]],
    )


def build_nc(with_bias=True):
    nc = bass.Bass()
    xt_d = nc.dram_tensor("xt", [D, N], BF16, kind="ExternalInput")
    wq_d = nc.dram_tensor("wq", [D, 3 * D], BF16, kind="ExternalInput")
    wo_d = nc.dram_tensor("wo", [D, D], BF16, kind="ExternalInput")
    bo_d = nc.dram_tensor("bo", [D], BF16, kind="ExternalInput")
    cos_d = nc.dram_tensor("cos2", [128, N], BF16, kind="ExternalInput")
    sin_d = nc.dram_tensor("sin2", [128, N], BF16, kind="ExternalInput")
    perm_d = nc.dram_tensor("perm", [128, 128], BF16, kind="ExternalInput")
    out_d = nc.dram_tensor("out", [D, N], F32, kind="ExternalOutput")
    import os as _os0

    _dbg = _os0.environ.get("K_DEBUG", "0") == "1"
    if _dbg:
        dbg_q = nc.dram_tensor("dbg_q", [128, KC, N], BF16, kind="ExternalOutput")
        dbg_k = nc.dram_tensor("dbg_k", [128, KC, N], BF16, kind="ExternalOutput")
        dbg_v = nc.dram_tensor(
            "dbg_v", [128, NK, H, DH + 1], BF16, kind="ExternalOutput"
        )
        dbg_au = nc.dram_tensor("dbg_au", [128, KC, N], F32, kind="ExternalOutput")
        dbg_sums = nc.dram_tensor("dbg_sums", [96, 128], F32, kind="ExternalOutput")
        dbg_attn = nc.dram_tensor("dbg_attn", [128, KC, N], BF16, kind="ExternalOutput")

    Exp = mybir.ActivationFunctionType.Exp
    Copy = mybir.ActivationFunctionType.Copy

    with tile_mod.TileContext(nc) as tc:
        with (
            tc.tile_pool(name="singles", bufs=1) as singles,
            tc.tile_pool(name="wq_pool", bufs=12) as wq_pool,
            tc.tile_pool(name="wo_pool", bufs=3) as wo_pool,
            tc.tile_pool(name="apool", bufs=4) as apool,
            tc.tile_pool(name="bpool", bufs=2) as bpool,
            tc.tile_pool(name="dpool", bufs=1, space="DRAM") as dpool,
        ):
            xt_sb = singles.tile([128, KC, N], BF16)
            wv_sb = singles.tile([128, KC, D], BF16)
            for kc in range(KC):
                nc.sync.dma_start(
                    out=xt_sb[:, kc, :], in_=xt_d[kc * 128 : (kc + 1) * 128, :]
                )
                nc.sync.dma_start(
                    out=wv_sb[:, kc, :],
                    in_=wq_d[kc * 128 : (kc + 1) * 128, 2 * D : 3 * D],
                )
            cos_sb = singles.tile([128, N], BF16)
            nc.sync.dma_start(out=cos_sb[:], in_=cos_d[:])
            sin_sb = singles.tile([128, N], BF16)
            nc.sync.dma_start(out=sin_sb[:], in_=sin_d[:])
            perm_sb = singles.tile([128, 128], BF16)
            nc.sync.dma_start(out=perm_sb[:], in_=perm_d[:])
            bo_sb = singles.tile([1, D], BF16)
            nc.sync.dma_start(
                out=bo_sb[:], in_=bo_d[:].rearrange("(o d) -> o d", o=1)
            )
            ones_sb = singles.tile([1, 512], BF16)
            nc.vector.memset(ones_sb[:], 1.0)
            wo_sb = singles.tile([128, KC, D], BF16)
            for c in range(KC):
                nc.sync.dma_start(
                    out=wo_sb[:, c, :], in_=wo_d[c * 128 : (c + 1) * 128, :]
                )

            v_sb = singles.tile([128, NK, H, DH + 1], BF16)
            nc.gpsimd.memset(v_sb[:, :, :, DH : DH + 1], 1.0)

            q_sb = singles.tile([128, KC, N], BF16)
            k_sb = singles.tile([128, KC, N], BF16)
            attnU_sb = singles.tile([128, KC, N], F32)
            attn_sb = singles.tile([128, KC, N], BF16)
            sums2_sb = singles.tile([96, 128], F32)
            recip2_sb = singles.tile([96, 128], F32)
            recip_d = dpool.tile([H * N], F32)
            recip_ap = recip_d[:]

            # ---- v projection: v[rows, 768] = x @ Wv -------------------
            with tc.tile_pool(name="ps_v", bufs=2, space="PSUM") as ps_v:
                for rc in range(RC):
                    vp = ps_v.tile([128, D], F32, tag="v")
                    for c0, w in ((0, 512), (512, 256)):
                        for kc in range(KC):
                            nc.tensor.matmul(
                                vp[:, c0 : c0 + w],
                                xt_sb[:, kc, rc * 128 : (rc + 1) * 128],
                                wv_sb[:, kc, c0 : c0 + w],
                                start=(kc == 0),
                                stop=(kc == KC - 1),
                            )
                    # strided copy into the [v | ones] per-head layout
                    nc.scalar.activation(
                        out=v_sb[:, rc, :, 0:DH],
                        in_=vp[:].rearrange("p (h d) -> p h d", h=H),
                        func=Copy,
                        scale=1.0,
                    )

            # ---- q^T / k^T projection + RoPE (one 128-col tile) --------
            def proj_oc(ps_qk, oc):
                col0 = oc * 128 if oc < KC else D + (oc - KC) * 128
                qkp = ps_qk.tile([128, N], F32, tag="qk", name=f"qkp{oc}")
                wts = []
                for kc in range(KC):
                    wt = wq_pool.tile([128, 128], BF16, tag="wq", name=f"wt{oc}_{kc}")
                    nc.sync.dma_start(
                        out=wt[:],
                        in_=wq_d[kc * 128 : (kc + 1) * 128, col0 : col0 + 128],
                    )
                    wts.append(wt)
                for qc in range(2):
                    for kc in range(KC):
                        nc.tensor.matmul(
                            qkp[:, qc * 512 : (qc + 1) * 512],
                            wts[kc][:],
                            xt_sb[:, kc, qc * 512 : (qc + 1) * 512],
                            start=(kc == 0),
                            stop=(kc == KC - 1),
                        )
                q0 = apool.tile([128, N], BF16, tag="q0", name=f"q0_{oc}")
                nc.vector.tensor_copy(q0[:], qkp[:])
                rotp = ps_qk.tile([128, N], F32, tag="qk", name=f"rotp{oc}")
                for qc in range(2):
                    nc.tensor.matmul(
                        rotp[:, qc * 512 : (qc + 1) * 512],
                        perm_sb[:],
                        q0[:, qc * 512 : (qc + 1) * 512],
                        start=True,
                        stop=True,
                    )
                t1 = apool.tile([128, N], BF16, tag="t1", name=f"t1_{oc}")
                nc.vector.tensor_mul(t1[:], rotp[:], sin_sb[:])
                t2 = apool.tile([128, N], BF16, tag="t2", name=f"t2_{oc}")
                nc.vector.tensor_mul(t2[:], q0[:], cos_sb[:])
                dst = q_sb if oc < KC else k_sb
                nc.vector.tensor_add(dst[:, oc % KC, :], t1[:], t2[:])

            # ---- attention: head pairs, row-packed K=64 S^T matmuls,
            # kc-paired psum tiles for wide exp, query-split for psum room
            def attn_pair(ps_att, qc, hp):
                # st tile per kc holds BOTH heads' scores side by side:
                # [128 keys, (head_a 512q | head_b 512q)] -> one wide exp
                pvs = []
                for a in range(2):
                    pv = ps_att.tile(
                        [65, 512], F32, tag=f"pv{a}", bufs=1, name=f"pv{a}_{qc}_{hp}"
                    )
                    pvs.append(pv)
                for kc in range(NK):
                    st = ps_att.tile(
                        [128, N], F32, tag="st", bufs=2, name=f"st_{qc}_{hp}_{kc}"
                    )
                    for a in range(2):
                        po = 64 * a
                        nc.tensor.matmul(
                            st[:, a * 512 : (a + 1) * 512],
                            k_sb[po : po + 64, hp, kc * 128 : (kc + 1) * 128],
                            q_sb[po : po + 64, hp, qc * 512 : (qc + 1) * 512],
                            start=True,
                            stop=True,
                        )
                    e = apool.tile([128, N], BF16, tag="e", name=f"e_{qc}_{hp}_{kc}")
                    nc.scalar.activation(out=e[:], in_=st[:], func=Exp, scale=SCALE)
                    for a in range(2):
                        nc.tensor.matmul(
                            pvs[a][:],
                            v_sb[:, kc, 2 * hp + a, :],
                            e[:, a * 512 : (a + 1) * 512],
                            start=(kc == 0),
                            stop=(kc == NK - 1),
                        )
                for a in range(2):
                    h = 2 * hp + a
                    po = 64 * a
                    pvt = bpool.tile(
                        [65, 512], F32, tag="pvt", name=f"pvt{qc}_{h}"
                    )
                    nc.vector.tensor_copy(pvt[:], pvs[a][:])
                    nc.sync.dma_start(
                        out=attnU_sb[po : po + 64, hp, qc * 512 : (qc + 1) * 512],
                        in_=pvt[0:64, :],
                    )
                    nc.sync.dma_start(
                        out=sums2_sb[h * 8 + qc * 4 : h * 8 + qc * 4 + 4, :],
                        in_=pvt[64:65, :],
                    )

            def normalize_quad(q4):
                nc.vector.reciprocal(
                    recip2_sb[q4 * 32 : (q4 + 1) * 32, :],
                    sums2_sb[q4 * 32 : (q4 + 1) * 32, :],
                )
                nc.sync.dma_start(
                    out=recip_d[:].rearrange("(p r) -> p r", p=96)[
                        q4 * 32 : (q4 + 1) * 32, :
                    ],
                    in_=recip2_sb[q4 * 32 : (q4 + 1) * 32, :],
                )
                for hp2 in (2 * q4, 2 * q4 + 1):
                    rb = bpool.tile([128, N], F32, tag="rb", name=f"rb{hp2}")
                    nc.sync.dma_start(
                        out=rb[0:64, :],
                        in_=_bcast_rows(recip_ap, (2 * hp2) * N, 64, N),
                    )
                    nc.sync.dma_start(
                        out=rb[64:128, :],
                        in_=_bcast_rows(recip_ap, (2 * hp2 + 1) * N, 64, N),
                    )
                    nc.vector.tensor_mul(
                        attn_sb[:, hp2, :], attnU_sb[:, hp2, :], rb[:]
                    )

            import os as _os

            _mode = _os.environ.get("K_MODE", "inter")
            if _mode == "v3a":
                # proj fully upfront in its own psum pool, then attention
                with tc.tile_pool(name="ps_qk", bufs=1, space="PSUM") as ps_qk:
                    for oc in range(KC):
                        proj_oc(ps_qk, oc)
                        proj_oc(ps_qk, KC + oc)
                with tc.tile_pool(name="ps_att", bufs=1, space="PSUM") as ps_att:
                    for hp in range(HP):
                        attn_pair(ps_att, 0, hp)
                    for hp in range(HP):
                        attn_pair(ps_att, 1, hp)
                        if hp % 2 == 1:
                            normalize_quad(hp // 2)
            else:
                with (
                    tc.tile_pool(name="ps_qk", bufs=1, space="PSUM") as ps_qk,
                    tc.tile_pool(name="ps_att", bufs=1, space="PSUM") as ps_att,
                ):
                    proj_oc(ps_qk, 0)
                    proj_oc(ps_qk, KC)
                    for hp in range(HP):
                        attn_pair(ps_att, 0, hp)
                        if hp + 1 < HP:
                            proj_oc(ps_qk, hp + 1)
                            proj_oc(ps_qk, KC + hp + 1)
                    for hp in range(HP):
                        attn_pair(ps_att, 1, hp)
                        if hp % 2 == 1:
                            normalize_quad(hp // 2)

            if _dbg:
                nc.sync.dma_start(out=dbg_q[:], in_=q_sb[:])
                nc.sync.dma_start(out=dbg_k[:], in_=k_sb[:])
                nc.sync.dma_start(out=dbg_v[:], in_=v_sb[:])
                nc.sync.dma_start(out=dbg_au[:], in_=attnU_sb[:])
                nc.sync.dma_start(out=dbg_sums[:], in_=sums2_sb[:])
                nc.sync.dma_start(out=dbg_attn[:], in_=attn_sb[:])

            # ---- out-projection ----------------------------------------
            with tc.tile_pool(name="ps_fin", bufs=3, space="PSUM") as ps_fin:
                for oc in range(KC):
                    fps = [
                        ps_fin.tile([128, 512], F32, tag="fin", name=f"fin{oc}_{i}")
                        for i in range(2)
                    ]
                    for c in range(KC):
                        for qc in range(2):
                            nc.tensor.matmul(
                                fps[qc][:],
                                wo_sb[:, c, oc * 128 : (oc + 1) * 128],
                                attn_sb[:, c, qc * 512 : (qc + 1) * 512],
                                start=(c == 0),
                                stop=(not with_bias and c == KC - 1),
                            )
                    for qc in range(2):
                        if with_bias:
                            nc.tensor.matmul(
                                fps[qc][:],
                                bo_sb[0:1, oc * 128 : (oc + 1) * 128],
                                ones_sb[:],
                                start=False,
                                stop=True,
                            )
                        fsb = bpool.tile([128, 512], F32, tag="fsb")
                        nc.scalar.activation(
                            out=fsb[:], in_=fps[qc][:], func=Copy, scale=1.0
                        )
                        nc.sync.dma_start(
                            out=out_d[
                                oc * 128 : (oc + 1) * 128, qc * 512 : (qc + 1) * 512
                            ],
                            in_=fsb[:],
                        )

    split_sync_waits(nc, max_waits=1)
    return nc


def _host_prep(x, w_qkv, w_out, b_out):
    bf = ml_dtypes.bfloat16
    inv_freq = 1.0 / (10000.0 ** (np.arange(0, DH, 2, dtype=np.float32) / DH))
    t = np.arange(N, dtype=np.float32)
    freqs = np.outer(t, inv_freq)
    emb = np.concatenate([freqs, freqs], axis=1)        # [N, DH]
    cos2 = np.tile(np.cos(emb).T.astype(np.float32), (2, 1)).astype(bf)
    sin2 = np.tile(np.sin(emb).T.astype(np.float32), (2, 1)).astype(bf)

    perm = np.zeros((128, 128), np.float32)
    for blk in range(2):
        o = blk * 64
        for m in range(32):
            perm[o + m + 32, o + m] = -1.0
        for m in range(32, 64):
            perm[o + m - 32, o + m] = 1.0
    perm = perm.astype(bf)

    xt = np.ascontiguousarray(x.transpose(0, 2, 1)).astype(bf)
    shared = {
        "wq": np.ascontiguousarray(w_qkv).astype(bf),
        "wo": np.ascontiguousarray(w_out).astype(bf),
        "bo": np.ascontiguousarray(b_out).astype(bf),
        "cos2": np.ascontiguousarray(cos2),
        "sin2": np.ascontiguousarray(sin2),
        "perm": np.ascontiguousarray(perm),
    }
    return [dict(shared, xt=np.ascontiguousarray(xt[i])) for i in range(B)]


_NC_CACHE = {}
LAST_EXEC_NS = [None]


def _run(in_maps, trace=False, with_bias=True):
    if with_bias not in _NC_CACHE:
        _NC_CACHE[with_bias] = build_nc(with_bias=with_bias)
    res = run_bass_kernel_spmd(
        _NC_CACHE[with_bias], in_maps, list(range(B)), trace=trace
    )
    LAST_EXEC_NS[0] = res.exec_time_ns
    out_t = np.stack([np.asarray(res.results[i]["out"]) for i in range(B)])
    return np.ascontiguousarray(out_t.transpose(0, 2, 1)).astype(np.float32)


def kernel(x, w_qkv, w_out, b_out, _trace=False):
    b_out = np.asarray(b_out, dtype=np.float32)
    in_maps = _host_prep(
        np.asarray(x, dtype=np.float32),
        np.asarray(w_qkv, dtype=np.float32),
        np.asarray(w_out, dtype=np.float32),
        b_out,
    )
    return _run(in_maps, trace=_trace, with_bias=bool(np.any(b_out)))
